# revision 2
# baseline (speedup 1.0000x reference)
"""Trainium2 Bass kernel for a causal single-head attention (B=16, S=2048, D=64).

Sharding: data-parallel over batch. 8 NeuronCores, 2 batches per core.
Per-core algorithm (all matmul compute in bf16, f32 accumulation):
  xT      = transpose(x) via bf16 cast + DRAM bounce + DMA xbar transpose
  qT/kT   = Wq_aug^T @ xT_aug   (bias folded in via ones row of xT_aug)
  scT     = kT_tile^T @ qT      (scores transposed: [k, q] tiles; the two
                                 batches run concurrently in the two halves
                                 of the PE array since contract dim is 64)
  E_T     = exp(scT / 8)        (ACT engine, PSUM -> SBUF bf16; causal mask
                                 on diagonal tiles via gpsimd affine_select)
  accT    = sum_k x_aug[k]^T E_T[k]   (ones column of x_aug -> rowsum row)
  out     = (accT^T @ Wv_aug2) * (1 / rowsum)   (Wv/bv applied at the end;
                                 rowsum transposed to per-partition layout
                                 via a single PE transpose per batch)
"""

import numpy as np
from contextlib import ExitStack

NB = 2  # batches per core
S = 2048
D = 64
P = 128
NT = S // P  # 16 s-tiles per batch
W = 512  # q-chunk width
NCH = S // W  # 4 q-chunks per batch
KPC = W // P  # 4 k-tiles per chunk
N_CORES = 8

_CACHE = {}


def _build_nc():
    import concourse.bass as bass
    import concourse.tile as tile
    from concourse import bacc, mybir
    from concourse.masks import make_identity

    f32 = mybir.dt.float32
    bf16 = mybir.dt.bfloat16
    AF = mybir.ActivationFunctionType
    ALU = mybir.AluOpType

    nc = bacc.Bacc(None, target_bir_lowering=False, debug=False)

    x_ext = nc.declare_dram_parameter("x", [NB, S, D], f32, isOutput=False)
    w_ext = {}
    for wname in ("Wq", "Wk", "Wv"):
        w_ext[wname] = nc.declare_dram_parameter(wname, [D, D], f32, isOutput=False)
    for bname in ("bq", "bk", "bv"):
        w_ext[bname] = nc.declare_dram_parameter(bname, [D], f32, isOutput=False)
    out_ext = nc.declare_dram_parameter("out", [NB, S, D], f32, isOutput=True)

    # DRAM bounce for the x transpose (cols padded to 128 for the xbar)
    xb_dram = nc.dram_tensor("xb_scratch", [NB, S, P], bf16)

    with ExitStack() as ctx:
        tc = ctx.enter_context(tile.TileContext(nc))

        singles = ctx.enter_context(tc.tile_pool(name="singles", bufs=1))
        xstage = ctx.enter_context(tc.tile_pool(name="xstage", bufs=2))
        etp = ctx.enter_context(tc.tile_pool(name="etp", bufs=4))
        outst = ctx.enter_context(tc.tile_pool(name="outst", bufs=4))
        psA = ctx.enter_context(
            tc.tile_pool(name="psA", bufs=2, space=bass.MemorySpace.PSUM)
        )
        psB = ctx.enter_context(
            tc.tile_pool(name="psB", bufs=1, space=bass.MemorySpace.PSUM)
        )
        psC = ctx.enter_context(
            tc.tile_pool(name="psC", bufs=2, space=bass.MemorySpace.PSUM)
        )

        # ---- constants: identity, augmented weights ----
        ident = singles.tile([P, P], bf16)
        make_identity(nc, ident)

        w_aug = {}
        for wname, bname in (("Wq", "bq"), ("Wk", "bk"), ("Wv", "bv")):
            aug = singles.tile([D + 1, D], bf16, name=f"{wname}_aug")
            wtmp = xstage.tile([D, D], f32, tag="wtmp")
            btmp = xstage.tile([1, D], f32, tag="btmp")
            nc.sync.dma_start(out=wtmp, in_=w_ext[wname].ap())
            nc.sync.dma_start(
                out=btmp, in_=w_ext[bname].ap().rearrange("(a d) -> a d", a=1)
            )
            nc.vector.tensor_copy(out=aug[0:D, :], in_=wtmp)
            nc.vector.tensor_copy(out=aug[D : D + 1, :], in_=btmp)
            w_aug[wname] = aug

        # ---- x load, bf16 cast (+ones col), transpose via DRAM bounce ----
        x_bf = []  # [128, NT, 65] natural bf16 tiles with ones column (av lhsT)
        xT_aug = []  # [128, 2048] bf16; rows 0..63 = xT, row 64 = ones
        for b in range(NB):
            xb = singles.tile([P, NT, D + 1], bf16, name=f"x_bf{b}")
            xt = singles.tile([P, S], bf16, name=f"xT_aug{b}")
            x_bf.append(xb)
            xT_aug.append(xt)

            xf = xstage.tile([P, NT, D], f32, tag="xf32")
            nc.sync.dma_start(
                out=xf, in_=x_ext.ap()[b].rearrange("(t p) d -> p t d", p=P)
            )
            nc.vector.tensor_copy(out=xb[:, :, 0:D], in_=xf)
            nc.vector.memset(xb[:, :, D : D + 1], 1.0)
            nc.sync.dma_start(
                out=xb_dram.ap()[b, :, 0 : D + 1].rearrange("(t p) e -> p t e", p=P),
                in_=xb,
            )
            nc.sync.dma_start_transpose(out=xt, in_=xb_dram.ap()[b])

        # ---- q/k projections: qT_all/kT_all [128, 2048] bf16 ----
        # partitions 0..63 = batch0, 64..127 = batch1 (enables row-packed scores)
        qT_all = singles.tile([P, S], bf16)
        kT_all = singles.tile([P, S], bf16)
        for c4 in range(NCH):
            cols = bass.ds(c4 * W, W)
            qp = psA.tile([P, W], f32, tag="sc")
            kp = psA.tile([P, W], f32, tag="sc")
            for b in range(NB):
                pr = bass.ds(b * D, D)
                nc.tensor.matmul(
                    qp[pr, :],
                    w_aug["Wq"],
                    xT_aug[b][0 : D + 1, cols],
                    tile_position=(0, b * D),
                )
                nc.tensor.matmul(
                    kp[pr, :],
                    w_aug["Wk"],
                    xT_aug[b][0 : D + 1, cols],
                    tile_position=(0, b * D),
                )
            # cast+move PSUM -> SBUF (split between ACT and DVE)
            nc.scalar.copy(out=qT_all[:, cols], in_=qp)
            nc.vector.tensor_copy(out=kT_all[:, cols], in_=kp)

        # ---- attention: per q-chunk, stream k-tiles ----
        acc_sbuf = []
        for b in range(NB):
            a = singles.tile([D + 1, NCH, W], bf16, name=f"acc_sbuf{b}")
            acc_sbuf.append(a)
        rowsum_resh = [
            singles.tile([NT, P], bf16, name=f"rowsum_resh{b}") for b in range(NB)
        ]

        for c in range(NCH):
            acc = [
                psB.tile([D + 1, W], f32, name=f"avacc{b}", tag=f"avacc{b}")
                for b in range(NB)
            ]
            nk = KPC * c + KPC
            for i in range(nk):
                off0 = max(0, P * i - W * c)
                span = W - off0
                q0 = W * c + off0

                sc = psA.tile([P, 2 * W], f32, tag="sc")
                # batch0 right-aligned in bank 0, batch1 left-aligned in bank 1
                for b in range(NB):
                    rows = bass.ds(b * D, D)
                    dst = sc[:, off0:W] if b == 0 else sc[:, W : W + span]
                    nc.tensor.matmul(
                        dst,
                        kT_all[rows, bass.ds(P * i, P)],
                        qT_all[rows, bass.ds(q0, span)],
                    )
                et = etp.tile([P, 2 * W], bf16, tag="et")
                nc.scalar.activation(
                    out=et[:, off0 : W + span],
                    in_=sc[:, off0 : W + span],
                    func=AF.Exp,
                    scale=0.125,
                )
                if i >= KPC * c:  # diagonal tile: causal mask (keep k <= q)
                    for b in range(NB):
                        reg = et[:, off0:W] if b == 0 else et[:, W : W + span]
                        nc.gpsimd.affine_select(
                            out=reg,
                            in_=reg,
                            base=0,
                            channel_multiplier=-1,
                            pattern=[[1, span]],
                            compare_op=ALU.is_ge,
                            fill=0.0,
                        )
                for b in range(NB):
                    reg = et[:, off0:W] if b == 0 else et[:, W : W + span]
                    nc.tensor.matmul(
                        acc[b][:, off0:W],
                        x_bf[b][:, i, :],
                        reg,
                        start=(i == 0),
                        stop=(i == nk - 1),
                    )
            for b in range(NB):
                nc.vector.tensor_copy(out=acc_sbuf[b][:, c, :], in_=acc[b])
                for jj in range(KPC):
                    nc.sync.dma_start(
                        out=rowsum_resh[b][KPC * c + jj : KPC * c + jj + 1, :],
                        in_=acc_sbuf[b][D : D + 1, c, bass.ds(P * jj, P)],
                    )

        # ---- epilogue: rowsum transpose + reciprocal, Wv proj, divide, out ----
        for b in range(NB):
            rsT = psC.tile([P, NT], bf16, tag="pc")
            nc.tensor.transpose(rsT, rowsum_resh[b], ident[0:NT, 0:NT])
            recip = singles.tile([P, NT], f32, name=f"recip{b}")
            nc.vector.reciprocal(out=recip, in_=rsT)

            for c in range(NCH):
                po = psC.tile([P, KPC, D], f32, tag="pc")
                for j in range(KPC):
                    nc.tensor.matmul(
                        po[:, j, :],
                        acc_sbuf[b][:, c, bass.ds(P * j, P)],
                        w_aug["Wv"],
                    )
                div = outst.tile([P, KPC, D], f32, tag="div")
                rc = recip[:, KPC * c : KPC * c + KPC]
                rc_b = bass.AP(
                    tensor=rc.tensor,
                    offset=rc.offset,
                    ap=[rc.ap[0], rc.ap[1], [0, D]],
                )
                nc.vector.tensor_mul(div, po, rc_b)
                nc.sync.dma_start(
                    out=out_ext.ap()[b, bass.ds(W * c, W), :].rearrange(
                        "(j p) d -> p j d", p=P
                    ),
                    in_=div,
                )

    nc.compile()
    return nc


def _get_nc():
    if "nc" not in _CACHE:
        _CACHE["nc"] = _build_nc()
    return _CACHE["nc"]


def kernel(**inputs) -> np.ndarray:
    from concourse.bass_utils import run_bass_kernel_spmd

    nc = _get_nc()
    x = np.ascontiguousarray(inputs["x"], dtype=np.float32)
    B = x.shape[0]
    assert B == NB * N_CORES
    reps = {
        k: np.ascontiguousarray(inputs[k], dtype=np.float32)
        for k in ("Wq", "bq", "Wk", "bk", "Wv", "bv")
    }
    in_maps = [
        {"x": np.ascontiguousarray(x[i * NB : (i + 1) * NB]), **reps}
        for i in range(N_CORES)
    ]
    res = run_bass_kernel_spmd(nc, in_maps, core_ids=list(range(N_CORES)))
    out = np.concatenate([res.results[i]["out"] for i in range(N_CORES)], axis=0)
    return out.astype(np.float32)
